# revision 40
# baseline (speedup 1.0000x reference)
"""Trainium2 Bass kernel for nn_MultiHeadAttention_ca (channel attention +
sync BatchNorm + ReLU + residual), data-parallel over batch on 8 NeuronCores.

Per sample (C=512 channels, S=128 spatial, M=512 middle):
    q = query @ Wq.T + bq            [C, M]
    k = key   @ Wk.T (+bk dropped: softmax is invariant to per-row shifts)
    v = value @ Wv.T (+bv folded into context: att rows sum to 1)
    att = softmax(q k^T / sqrt(M))   [C, C]   (output 1)
    ctx = att @ v + bv               [C, M]
    out = ctx @ Wf.T + bf            [C, S]
    out = relu(BN_{batch,spatial}(out) * gamma + beta) + value   (output 2)

Layout strategy (everything 128-partition tiles, all matmuls N=512 f32r):
  host pre-transposes query/key/value to [S, C] so projections need no
  on-device input transposes; att is softmaxed in natural [i, j] layout
  (free-dim reductions + direct output DMA) and PE-transposed (16 blocks)
  for the att @ v contraction. BN stats via bn_stats/bn_aggr, synced with
  one 4KB AllReduce across the 8 cores.
"""
import os
import sys
import numpy as np
from contextlib import ExitStack

sys.path.insert(0, "/opt/trn_rl_repo")

import concourse.bass as bass
import concourse.bacc as bacc
import concourse.tile as tile
from concourse import mybir
from concourse.masks import make_identity
from concourse.bass_utils import run_bass_kernel_spmd

P = 128          # partitions
B = 128          # batch
C = 512          # channels (attention runs over channel dim)
S = 128          # spatial
M = 512          # middle
NC_ = C // P     # 4 channel chunks
NM = M // P      # 4 middle chunks
N_CORES = 8
BPC = B // N_CORES   # 16 samples per core
GRP = 2              # samples per input-load group
SCALE = 1.0 / float(np.sqrt(M))
BN_EPS = 1e-5
BN_N = B * S         # 16384 elements per channel for BN stats

f32 = mybir.dt.float32
f32r = mybir.dt.float32r
bf16 = mybir.dt.bfloat16
AF = mybir.ActivationFunctionType
ALU = mybir.AluOpType

# context path (att @ v) in bf16: E is cast to bf16 and transposed on the
# DMA xbar instead of the PE; v is stored bf16. Attention output precision
# is unaffected (normalize runs on the fp32 E); only `out` drops to ~1e-3.
CTX_BF16 = False

_CACHE: dict = {}


def build_program(n_cores: int = N_CORES, bpc: int = BPC, use_cc: bool = True,
                  repeat: int = 1, internal_io: bool = False):
    """repeat/internal_io are for timing experiments only: repeat re-runs the
    sample loop R times in-program; internal_io makes the big outputs
    internal DRAM (tiny dummy external output) to cut RPC bookkeeping."""
    nc = bacc.Bacc(None, num_devices=n_cores)

    # ---- per-core DRAM I/O ----
    qT_d = nc.dram_tensor("qT", [bpc, S, C], f32r, kind="ExternalInput")
    kT_d = nc.dram_tensor("kT", [bpc, S, C], f32r, kind="ExternalInput")
    vT_d = nc.dram_tensor("vT", [bpc, S, C], f32r, kind="ExternalInput")
    vnat_d = nc.dram_tensor("vnat", [bpc, C, S], f32, kind="ExternalInput")
    WqT_d = nc.dram_tensor("WqT", [S, M], f32r, kind="ExternalInput")
    WkT_d = nc.dram_tensor("WkT", [S, M], f32r, kind="ExternalInput")
    WvT_d = nc.dram_tensor("WvT", [S, M], f32r, kind="ExternalInput")
    WfT_d = nc.dram_tensor("WfT", [M, S], f32r, kind="ExternalInput")
    bq_d = nc.dram_tensor("bq_r", [P, NM], f32, kind="ExternalInput")
    bf_d = nc.dram_tensor("bf2", [S], f32, kind="ExternalInput")
    gamma_d = nc.dram_tensor("gamma_r", [P, NC_], f32, kind="ExternalInput")
    beta_d = nc.dram_tensor("beta_r", [P, NC_], f32, kind="ExternalInput")

    if internal_io:
        att_d = nc.dram_tensor("att_i", [bpc, C, C], f32)
        out_d = nc.dram_tensor("out_i", [bpc, C, S], f32)
        dummy_d = nc.dram_tensor("dummy_o", [P, S], f32, kind="ExternalOutput")
    else:
        att_d = nc.dram_tensor("att_o", [bpc, C, C], f32, kind="ExternalOutput")
        out_d = nc.dram_tensor("out_o", [bpc, C, S], f32, kind="ExternalOutput")

    with tile.TileContext(nc) as tc, ExitStack() as ctx:
        const = ctx.enter_context(tc.tile_pool(name="const", bufs=1))
        gin = ctx.enter_context(tc.tile_pool(name="gin", bufs=2))
        big = ctx.enter_context(tc.tile_pool(name="big", bufs=1))
        work2 = ctx.enter_context(tc.tile_pool(name="work2", bufs=2))
        work1 = ctx.enter_context(tc.tile_pool(name="work1", bufs=1))
        small = ctx.enter_context(tc.tile_pool(name="small", bufs=2))
        fin = ctx.enter_context(tc.tile_pool(name="fin", bufs=1))
        ps2 = ctx.enter_context(tc.tile_pool(name="ps2", bufs=2, space="PSUM"))
        dram = ctx.enter_context(tc.tile_pool(name="dram", bufs=1, space="DRAM"))

        # ---- constants ----
        WqT = const.tile([S, M], f32r)
        WkT = const.tile([S, M], f32r)
        WvT = const.tile([S, M], f32r)
        WfT = const.tile([P, NM, S], f32r)     # [m%128, m-chunk, s]
        nc.sync.dma_start(out=WqT, in_=WqT_d[:, :])
        nc.sync.dma_start(out=WkT, in_=WkT_d[:, :])
        nc.sync.dma_start(out=WvT, in_=WvT_d[:, :])
        nc.sync.dma_start(out=WfT, in_=WfT_d.rearrange("(k p) s -> p k s", p=P))
        bq_r = const.tile([P, NM], f32)
        gamma_r = const.tile([P, NC_], f32)
        beta_r = const.tile([P, NC_], f32)
        nc.sync.dma_start(out=bq_r, in_=bq_d[:, :])
        nc.sync.dma_start(out=gamma_r, in_=gamma_d[:, :])
        nc.sync.dma_start(out=beta_r, in_=beta_d[:, :])
        bf_bc = const.tile([P, S], f32)
        bf_ap = bf_d[:]
        nc.sync.dma_start(
            out=bf_bc,
            in_=bass.AP(tensor=bf_ap.tensor, offset=bf_ap.offset,
                        ap=[[0, P], [1, S]]))
        ident = const.tile([P, P], f32)
        make_identity(nc, ident)

        # ---- big persistent tiles ----
        # value natural + held pre-BN output, layout [p, c-chunk, b, s]
        # (vnat DMA is issued late, after the sample loop — it is only read
        # by the finale and must not delay the first input group loads)
        vnat = big.tile([P, NC_, bpc, S], f32)
        hold = big.tile([P, NC_, bpc, S], f32)
        stats = big.tile([P, NC_, bpc, 6], f32)

        # ---- per-sample pipeline ----
        for bb in range(bpc * repeat):
            b = bb % bpc
            g0 = (b // GRP) * GRP
            if b % GRP == 0:
                qT_g = gin.tile([S, GRP, C], f32r, name="qT_g")
                kT_g = gin.tile([S, GRP, C], f32r, name="kT_g")
                vT_g = gin.tile([S, GRP, C], f32r, name="vT_g")
                nc.sync.dma_start(
                    out=qT_g, in_=qT_d[g0:g0 + GRP].rearrange("b s c -> s b c"))
                nc.sync.dma_start(
                    out=kT_g, in_=kT_d[g0:g0 + GRP].rearrange("b s c -> s b c"))
                nc.sync.dma_start(
                    out=vT_g, in_=vT_d[g0:g0 + GRP].rearrange("b s c -> s b c"))
            bg = b - g0

            # projections: qT/kT [m%128, m-chunk, c],  v [c%128, c-chunk, m]
            qT_sb = work2.tile([P, NM, C], f32r, name="qT_sb")
            kT_sb = work2.tile([P, NM, C], f32r, name="kT_sb")
            v_sb = work2.tile([P, NC_, M], bf16 if CTX_BF16 else f32r,
                              name="v_sb")
            for mc in range(NM):
                pps = ps2.tile([P, C], f32, tag="proj", name="pps")
                nc.tensor.matmul(pps, lhsT=WqT[:, mc * P:(mc + 1) * P],
                                 rhs=qT_g[:, bg, :], start=True, stop=True)
                nc.vector.tensor_scalar(out=qT_sb[:, mc, :], in0=pps,
                                        scalar1=bq_r[:, mc:mc + 1], scalar2=None,
                                        op0=ALU.add)
            for mc in range(NM):
                pps = ps2.tile([P, C], f32, tag="proj", name="pps")
                nc.tensor.matmul(pps, lhsT=WkT[:, mc * P:(mc + 1) * P],
                                 rhs=kT_g[:, bg, :], start=True, stop=True)
                nc.scalar.activation(out=kT_sb[:, mc, :], in_=pps, func=AF.Copy)
            for cc in range(NC_):
                pps = ps2.tile([P, M], f32, tag="proj", name="pps")
                nc.tensor.matmul(pps, lhsT=vT_g[:, bg, cc * P:(cc + 1) * P],
                                 rhs=WvT[:, :], start=True, stop=True)
                if CTX_BF16 and cc < 2:
                    nc.scalar.activation(out=v_sb[:, cc, :], in_=pps,
                                         func=AF.Copy)
                else:
                    nc.vector.tensor_copy(out=v_sb[:, cc, :], in_=pps)

            # scores -> E = exp(scale*scores) (natural [i, j]); denominators
            # ride the exp's accum_out and are applied later in the
            # out-projection epilogue (per-partition there), so the PE can
            # transpose unnormalized E without waiting on DVE.
            att_sb = work2.tile([P, NC_, C], f32, name="att_sb")
            den = small.tile([P, NC_], f32, name="den")
            rden = small.tile([P, NC_], f32, name="rden")
            for ic in range(NC_):
                sps = ps2.tile([P, C], f32, tag="scores", bufs=2, name="sps")
                for mc in range(NM):
                    nc.tensor.matmul(sps,
                                     lhsT=qT_sb[:, mc, ic * P:(ic + 1) * P],
                                     rhs=kT_sb[:, mc, :],
                                     start=(mc == 0), stop=(mc == NM - 1))
                nc.scalar.activation(out=att_sb[:, ic, :], in_=sps, func=AF.Exp,
                                     scale=SCALE, accum_out=den[:, ic:ic + 1])
            nc.vector.reciprocal(out=rden, in_=den)

            # transpose E -> ET [j%128, j-chunk, i]
            if CTX_BF16:
                # cast E to bf16, transpose 128x128 blocks on the DMA xbar
                att_bf = work2.tile([P, NC_, C], bf16, name="att_bf")
                for ic in range(NC_):
                    nc.vector.tensor_copy(out=att_bf[:, ic, :],
                                          in_=att_sb[:, ic, :])
                attT_sb = work1.tile([P, NC_, C], bf16, name="attT_sb")
                for jc in range(NC_):
                    for ic in range(NC_):
                        nc.scalar.dma_start_transpose(
                            out=attT_sb[:, jc, ic * P:(ic + 1) * P],
                            in_=att_bf[:, ic, jc * P:(jc + 1) * P])
            else:
                # 16 PE transpose blocks through PSUM
                attT_sb = work1.tile([P, NC_, C], f32r, name="attT_sb")
                for jc in range(NC_):
                    tps = ps2.tile([P, C], f32, tag="attT", bufs=2, name="tps")
                    for ic in range(NC_):
                        nc.tensor.transpose(tps[:, ic * P:(ic + 1) * P],
                                            att_sb[:, ic, jc * P:(jc + 1) * P],
                                            ident)
                    nc.scalar.activation(out=attT_sb[:, jc, :], in_=tps,
                                         func=AF.Copy)

            # normalize att for the attention output (off the PE path)
            for ic in range(NC_):
                nc.vector.tensor_scalar(out=att_sb[:, ic, :],
                                        in0=att_sb[:, ic, :],
                                        scalar1=rden[:, ic:ic + 1], scalar2=None,
                                        op0=ALU.mult)
            nc.scalar.dma_start(
                out=att_d[b].rearrange("(k p) j -> p k j", p=P), in_=att_sb)

            # context^T [m%128, m-chunk, i] = v^T E^T (denominator deferred)
            ctxT_sb = work1.tile([P, NM, C], f32r, name="ctxT_sb")
            for mc in range(NM):
                cps = ps2.tile([P, C], f32, tag="ctx", name="cps")
                for jc in range(NC_):
                    nc.tensor.matmul(cps,
                                     lhsT=v_sb[:, jc, mc * P:(mc + 1) * P],
                                     rhs=attT_sb[:, jc, :],
                                     start=(jc == 0), stop=(jc == NC_ - 1))
                nc.scalar.activation(out=ctxT_sb[:, mc, :], in_=cps,
                                     func=AF.Copy)

            # out projection (natural [i%128, s]; one PSUM bank, 4 slices);
            # epilogue applies softmax denominator + bf2 (= bf + Wf@bv)
            ops = ps2.tile([P, NC_ * S], f32, tag="ctx", name="ops")
            for ic in range(NC_):
                for mc in range(NM):
                    nc.tensor.matmul(ops[:, ic * S:(ic + 1) * S],
                                     lhsT=ctxT_sb[:, mc, ic * P:(ic + 1) * P],
                                     rhs=WfT[:, mc, :],
                                     start=(mc == 0), stop=(mc == NM - 1))
            for ic in range(NC_):
                nc.vector.scalar_tensor_tensor(out=hold[:, ic, b, :],
                                               in0=ops[:, ic * S:(ic + 1) * S],
                                               scalar=rden[:, ic:ic + 1],
                                               in1=bf_bc,
                                               op0=ALU.mult, op1=ALU.add)
                nc.vector.bn_stats(out=stats[:, ic, b, :],
                                   in_=hold[:, ic, b, :])

        # vnat load (needed only by the finale; emitted late so it never
        # delays the input-group loads on the sync ring)
        vnat_r = vnat_d.rearrange("b (k p) s -> p k b s", p=P)
        for ic in range(NC_):
            nc.sync.dma_start(out=vnat[:, ic, :, :], in_=vnat_r[:, ic, :, :])

        # ---- sync BN ----
        mv = fin.tile([P, NC_, 2], f32)
        for ic in range(NC_):
            nc.vector.bn_aggr(out=mv[:, ic, :], in_=stats[:, ic, :, :])
        sums = fin.tile([P, 2, NC_], f32)
        tmp = fin.tile([P, NC_], f32)
        # sums[0] = mean * n_local ; sums[1] = (var + mean^2) * n_local
        n_local = float(bpc * S)
        nc.vector.tensor_scalar(out=sums[:, 0, :], in0=mv[:, :, 0],
                                scalar1=n_local, scalar2=None, op0=ALU.mult)
        nc.vector.tensor_mul(tmp, mv[:, :, 0], mv[:, :, 0])
        nc.vector.tensor_add(tmp, tmp, mv[:, :, 1])
        nc.vector.tensor_scalar(out=sums[:, 1, :], in0=tmp,
                                scalar1=n_local, scalar2=None, op0=ALU.mult)

        gsums = fin.tile([P, 2, NC_], f32)
        if use_cc:
            cc_in = dram.tile([P, 2 * NC_], f32)
            cc_out = dram.tile([P, 2 * NC_], f32)
            nc.gpsimd.dma_start(out=cc_in, in_=sums.rearrange("p a k -> p (a k)"))
            nc.gpsimd.collective_compute(
                "AllReduce", ALU.add,
                replica_groups=[list(range(n_cores))],
                ins=[cc_in.opt()], outs=[cc_out.opt()])
            nc.gpsimd.dma_start(out=gsums.rearrange("p a k -> p (a k)"),
                                in_=cc_out)
        else:
            nc.vector.tensor_copy(out=gsums, in_=sums)

        n_glob = float(BN_N) if use_cc else n_local
        mean_g = fin.tile([P, NC_], f32)
        ex2 = fin.tile([P, NC_], f32)
        var_g = fin.tile([P, NC_], f32)
        rstd = fin.tile([P, NC_], f32)
        a_sc = fin.tile([P, NC_], f32)
        b_sc = fin.tile([P, NC_], f32)
        nc.vector.tensor_scalar(out=mean_g, in0=gsums[:, 0, :],
                                scalar1=1.0 / n_glob, scalar2=None, op0=ALU.mult)
        nc.vector.tensor_scalar(out=ex2, in0=gsums[:, 1, :],
                                scalar1=1.0 / n_glob, scalar2=None, op0=ALU.mult)
        nc.vector.tensor_mul(var_g, mean_g, mean_g)
        nc.vector.tensor_sub(var_g, ex2, var_g)
        eps_t = fin.tile([P, 1], f32)
        nc.vector.memset(eps_t, BN_EPS)
        nc.scalar.activation(out=rstd, in_=var_g, func=AF.Sqrt, bias=eps_t)
        nc.vector.reciprocal(out=rstd, in_=rstd)
        nc.vector.tensor_mul(a_sc, gamma_r, rstd)
        nc.vector.tensor_mul(b_sc, mean_g, a_sc)
        nc.vector.tensor_sub(b_sc, beta_r, b_sc)

        # ---- finalize: relu(x*A + B) + value, then store ----
        for ic in range(NC_):
            nc.scalar.activation(out=hold[:, ic, :, :], in_=hold[:, ic, :, :],
                                 func=AF.Relu, scale=a_sc[:, ic:ic + 1],
                                 bias=b_sc[:, ic:ic + 1])
            nc.vector.tensor_add(hold[:, ic, :, :], hold[:, ic, :, :],
                                 vnat[:, ic, :, :])
        out_r = out_d.rearrange("b (k p) s -> p k b s", p=P)
        for ic in range(NC_):
            nc.scalar.dma_start(out=out_r[:, ic, :, :], in_=hold[:, ic, :, :])
        if internal_io:
            nc.scalar.dma_start(out=dummy_d[:, :], in_=hold[:, 0, 0, :])

    nc.compile()
    return nc


def _prep_host(inputs):
    """Host-side sharding prep: transposes + small-param relayouts."""
    q = np.ascontiguousarray(inputs["query"], dtype=np.float32)
    k = np.ascontiguousarray(inputs["key"], dtype=np.float32)
    v = np.ascontiguousarray(inputs["value"], dtype=np.float32)
    qT = np.ascontiguousarray(q.transpose(0, 2, 1))
    kT = np.ascontiguousarray(k.transpose(0, 2, 1))
    vT = np.ascontiguousarray(v.transpose(0, 2, 1))
    WqT = np.ascontiguousarray(np.asarray(inputs["Wq"], np.float32).T)
    WkT = np.ascontiguousarray(np.asarray(inputs["Wk"], np.float32).T)
    WvT = np.ascontiguousarray(np.asarray(inputs["Wv"], np.float32).T)
    WfT = np.ascontiguousarray(np.asarray(inputs["Wf"], np.float32).T)
    bq_r = np.ascontiguousarray(
        np.asarray(inputs["bq"], np.float32).reshape(NM, P).T)
    gamma_r = np.ascontiguousarray(
        np.asarray(inputs["gamma"], np.float32).reshape(NC_, P).T)
    beta_r = np.ascontiguousarray(
        np.asarray(inputs["beta"], np.float32).reshape(NC_, P).T)
    # bv folded through attention (rows sum to 1): bf2 = bf + Wf @ bv
    Wf = np.asarray(inputs["Wf"], np.float32)
    bv = np.asarray(inputs["bv"], np.float32)
    bf2 = np.ascontiguousarray(
        np.asarray(inputs["bf"], np.float32) + Wf @ bv)
    shared = dict(WqT=WqT, WkT=WkT, WvT=WvT, WfT=WfT, bq_r=bq_r,
                  bf2=bf2, gamma_r=gamma_r, beta_r=beta_r)
    in_maps = []
    for c in range(N_CORES):
        sl = slice(c * BPC, (c + 1) * BPC)
        m = dict(shared)
        m["qT"] = qT[sl]
        m["kT"] = kT[sl]
        m["vT"] = vT[sl]
        m["vnat"] = v[sl]
        in_maps.append(m)
    return in_maps


def kernel(**inputs):
    if "nc" not in _CACHE:
        _CACHE["nc"] = build_program()
    nc = _CACHE["nc"]
    in_maps = _prep_host(inputs)
    trace = bool(int(os.environ.get("KERNEL_TRACE", "0")))
    res = run_bass_kernel_spmd(nc, in_maps, list(range(N_CORES)), trace=trace)
    _CACHE["last_result"] = res
    out_full = np.concatenate([r["out_o"] for r in res.results], axis=0)
    att_full = np.concatenate([r["att_o"] for r in res.results], axis=0)
    return out_full, att_full


# revision 49
# speedup vs baseline: 2.7144x; 2.7144x over previous
"""Trainium2 Bass kernel for nn_MultiHeadAttention_ca (channel attention +
sync BatchNorm + ReLU + residual), data-parallel over batch on 8 NeuronCores.

Per sample (C=512 channels, S=128 spatial, M=512 middle):
    q = query @ Wq.T + bq            [C, M]
    k = key   @ Wk.T (+bk dropped: softmax is invariant to per-row shifts)
    v = value @ Wv.T (+bv folded into context: att rows sum to 1)
    att = softmax(q k^T / sqrt(M))   [C, C]   (output 1)
    ctx = att @ v + bv               [C, M]
    out = ctx @ Wf.T + bf            [C, S]
    out = relu(BN_{batch,spatial}(out) * gamma + beta) + value   (output 2)

Layout strategy (everything 128-partition tiles, all matmuls N=512 f32r):
  host pre-transposes query/key/value to [S, C] so projections need no
  on-device input transposes; att is softmaxed in natural [i, j] layout
  (free-dim reductions + direct output DMA) and PE-transposed (16 blocks)
  for the att @ v contraction. BN stats via bn_stats/bn_aggr, synced with
  one 4KB AllReduce across the 8 cores.
"""
import os
import sys
import numpy as np
from contextlib import ExitStack

sys.path.insert(0, "/opt/trn_rl_repo")

import concourse.bass as bass
import concourse.bacc as bacc
import concourse.tile as tile
from concourse import mybir
from concourse.masks import make_identity
from concourse.bass_utils import run_bass_kernel_spmd

P = 128          # partitions
B = 128          # batch
C = 512          # channels (attention runs over channel dim)
S = 128          # spatial
M = 512          # middle
NC_ = C // P     # 4 channel chunks
NM = M // P      # 4 middle chunks
N_CORES = 8
BPC = B // N_CORES   # 16 samples per core
GRP = 2              # samples per input-load group
SCALE = 1.0 / float(np.sqrt(M))
BN_EPS = 1e-5
BN_N = B * S         # 16384 elements per channel for BN stats

f32 = mybir.dt.float32
f32r = mybir.dt.float32r
bf16 = mybir.dt.bfloat16
AF = mybir.ActivationFunctionType
ALU = mybir.AluOpType

# context path (att @ v) in bf16: E is cast to bf16 and transposed on the
# DMA xbar instead of the PE; v is stored bf16. Attention output precision
# is unaffected (normalize runs on the fp32 E); only `out` drops to ~1e-3.
CTX_BF16 = False

_CACHE: dict = {}


def build_program(n_cores: int = N_CORES, bpc: int = BPC, use_cc: bool = True,
                  repeat: int = 1, internal_io: bool = False):
    """repeat/internal_io are for timing experiments only: repeat re-runs the
    sample loop R times in-program; internal_io makes the big outputs
    internal DRAM (tiny dummy external output) to cut RPC bookkeeping."""
    nc = bacc.Bacc(None, num_devices=n_cores)

    # ---- per-core DRAM I/O ----
    qT_d = nc.dram_tensor("qT", [bpc, S, C], f32r, kind="ExternalInput")
    kT_d = nc.dram_tensor("kT", [bpc, S, C], f32r, kind="ExternalInput")
    vT_d = nc.dram_tensor("vT", [bpc, S, C], f32r, kind="ExternalInput")
    vnat_d = nc.dram_tensor("vnat", [bpc, C, S], f32, kind="ExternalInput")
    WqT_d = nc.dram_tensor("WqT", [S, M], f32r, kind="ExternalInput")
    WkT_d = nc.dram_tensor("WkT", [S, M], f32r, kind="ExternalInput")
    WvT_d = nc.dram_tensor("WvT", [S, M], f32r, kind="ExternalInput")
    WfT_d = nc.dram_tensor("WfT", [M, S], f32r, kind="ExternalInput")
    bq_d = nc.dram_tensor("bq_r", [P, NM], f32, kind="ExternalInput")
    bf_d = nc.dram_tensor("bf2", [S], f32, kind="ExternalInput")
    gamma_d = nc.dram_tensor("gamma_r", [P, NC_], f32, kind="ExternalInput")
    beta_d = nc.dram_tensor("beta_r", [P, NC_], f32, kind="ExternalInput")

    if internal_io:
        att_d = nc.dram_tensor("att_i", [bpc, C, C], f32)
        out_d = nc.dram_tensor("out_i", [bpc, C, S], f32)
        dummy_d = nc.dram_tensor("dummy_o", [P, S], f32, kind="ExternalOutput")
    else:
        att_d = nc.dram_tensor("att_o", [bpc, C, C], f32, kind="ExternalOutput")
        out_d = nc.dram_tensor("out_o", [bpc, C, S], f32, kind="ExternalOutput")

    with tile.TileContext(nc) as tc, ExitStack() as ctx:
        const = ctx.enter_context(tc.tile_pool(name="const", bufs=1))
        gin = ctx.enter_context(tc.tile_pool(name="gin", bufs=2))
        big = ctx.enter_context(tc.tile_pool(name="big", bufs=1))
        work2 = ctx.enter_context(tc.tile_pool(name="work2", bufs=2))
        work1 = ctx.enter_context(tc.tile_pool(name="work1", bufs=1))
        small = ctx.enter_context(tc.tile_pool(name="small", bufs=2))
        fin = ctx.enter_context(tc.tile_pool(name="fin", bufs=1))
        ps2 = ctx.enter_context(tc.tile_pool(name="ps2", bufs=2, space="PSUM"))
        dram = ctx.enter_context(tc.tile_pool(name="dram", bufs=1, space="DRAM"))

        # ---- constants ----
        WqT = const.tile([S, M], f32r)
        WkT = const.tile([S, M], f32r)
        WvT = const.tile([S, M], f32r)
        WfT = const.tile([P, NM, S], f32r)     # [m%128, m-chunk, s]
        nc.sync.dma_start(out=WqT, in_=WqT_d[:, :])
        nc.sync.dma_start(out=WkT, in_=WkT_d[:, :])
        nc.sync.dma_start(out=WvT, in_=WvT_d[:, :])
        nc.sync.dma_start(out=WfT, in_=WfT_d.rearrange("(k p) s -> p k s", p=P))
        bq_r = const.tile([P, NM], f32)
        gamma_r = const.tile([P, NC_], f32)
        beta_r = const.tile([P, NC_], f32)
        nc.sync.dma_start(out=bq_r, in_=bq_d[:, :])
        nc.sync.dma_start(out=gamma_r, in_=gamma_d[:, :])
        nc.sync.dma_start(out=beta_r, in_=beta_d[:, :])
        bf_bc = const.tile([P, S], f32)
        bf_ap = bf_d[:]
        nc.sync.dma_start(
            out=bf_bc,
            in_=bass.AP(tensor=bf_ap.tensor, offset=bf_ap.offset,
                        ap=[[0, P], [1, S]]))
        ident = const.tile([P, P], f32)
        make_identity(nc, ident)

        # ---- big persistent tiles ----
        # value natural + held pre-BN output, layout [p, c-chunk, b, s]
        # (vnat DMA is issued late, after the sample loop — it is only read
        # by the finale and must not delay the first input group loads)
        vnat = big.tile([P, NC_, bpc, S], f32)
        hold = big.tile([P, NC_, bpc, S], f32)
        stats = big.tile([P, NC_, bpc, 6], f32)

        # ---- per-sample pipeline ----
        for bb in range(bpc * repeat):
            b = bb % bpc
            g0 = (b // GRP) * GRP
            if b % GRP == 0:
                qT_g = gin.tile([S, GRP, C], f32r, name="qT_g")
                kT_g = gin.tile([S, GRP, C], f32r, name="kT_g")
                vT_g = gin.tile([S, GRP, C], f32r, name="vT_g")
                nc.sync.dma_start(
                    out=qT_g, in_=qT_d[g0:g0 + GRP].rearrange("b s c -> s b c"))
                nc.sync.dma_start(
                    out=kT_g, in_=kT_d[g0:g0 + GRP].rearrange("b s c -> s b c"))
                nc.sync.dma_start(
                    out=vT_g, in_=vT_d[g0:g0 + GRP].rearrange("b s c -> s b c"))
            bg = b - g0

            # projections: qT/kT [m%128, m-chunk, c],  v [c%128, c-chunk, m]
            qT_sb = work2.tile([P, NM, C], f32r, name="qT_sb")
            kT_sb = work2.tile([P, NM, C], f32r, name="kT_sb")
            v_sb = work2.tile([P, NC_, M], bf16 if CTX_BF16 else f32r,
                              name="v_sb")
            for mc in range(NM):
                pps = ps2.tile([P, C], f32, tag="proj", name="pps")
                nc.tensor.matmul(pps, lhsT=WqT[:, mc * P:(mc + 1) * P],
                                 rhs=qT_g[:, bg, :], start=True, stop=True)
                nc.vector.tensor_scalar(out=qT_sb[:, mc, :], in0=pps,
                                        scalar1=bq_r[:, mc:mc + 1], scalar2=None,
                                        op0=ALU.add)
            for mc in range(NM):
                pps = ps2.tile([P, C], f32, tag="proj", name="pps")
                nc.tensor.matmul(pps, lhsT=WkT[:, mc * P:(mc + 1) * P],
                                 rhs=kT_g[:, bg, :], start=True, stop=True)
                nc.scalar.activation(out=kT_sb[:, mc, :], in_=pps, func=AF.Copy)
            for cc in range(NC_):
                pps = ps2.tile([P, M], f32, tag="proj", name="pps")
                nc.tensor.matmul(pps, lhsT=vT_g[:, bg, cc * P:(cc + 1) * P],
                                 rhs=WvT[:, :], start=True, stop=True)
                if CTX_BF16 and cc < 2:
                    nc.scalar.activation(out=v_sb[:, cc, :], in_=pps,
                                         func=AF.Copy)
                else:
                    nc.vector.tensor_copy(out=v_sb[:, cc, :], in_=pps)

            # scores -> E = exp(scale*scores) (natural [i, j]); denominators
            # ride the exp's accum_out and are applied later in the
            # out-projection epilogue (per-partition there), so the PE can
            # transpose unnormalized E without waiting on DVE.
            att_sb = work2.tile([P, NC_, C], f32, name="att_sb")
            den = small.tile([P, NC_], f32, bufs=4, name="den")
            rden = small.tile([P, NC_], f32, bufs=4, name="rden")
            for ic in range(NC_):
                sps = ps2.tile([P, C], f32, tag="scores", bufs=2, name="sps")
                for mc in range(NM):
                    nc.tensor.matmul(sps,
                                     lhsT=qT_sb[:, mc, ic * P:(ic + 1) * P],
                                     rhs=kT_sb[:, mc, :],
                                     start=(mc == 0), stop=(mc == NM - 1))
                nc.scalar.activation(out=att_sb[:, ic, :], in_=sps, func=AF.Exp,
                                     scale=SCALE, accum_out=den[:, ic:ic + 1])
            nc.vector.reciprocal(out=rden, in_=den)

            # transpose E -> ET [j%128, j-chunk, i]
            if CTX_BF16:
                # cast E to bf16, transpose 128x128 blocks on the DMA xbar
                att_bf = work2.tile([P, NC_, C], bf16, name="att_bf")
                for ic in range(NC_):
                    nc.vector.tensor_copy(out=att_bf[:, ic, :],
                                          in_=att_sb[:, ic, :])
                attT_sb = work1.tile([P, NC_, C], bf16, name="attT_sb")
                for jc in range(NC_):
                    for ic in range(NC_):
                        nc.scalar.dma_start_transpose(
                            out=attT_sb[:, jc, ic * P:(ic + 1) * P],
                            in_=att_bf[:, ic, jc * P:(jc + 1) * P])
            else:
                # 16 PE transpose blocks through PSUM
                attT_sb = work1.tile([P, NC_, C], f32r, name="attT_sb")
                for jc in range(NC_):
                    tps = ps2.tile([P, C], f32, tag="attT", bufs=2, name="tps")
                    for ic in range(NC_):
                        nc.tensor.transpose(tps[:, ic * P:(ic + 1) * P],
                                            att_sb[:, ic, jc * P:(jc + 1) * P],
                                            ident)
                    nc.scalar.activation(out=attT_sb[:, jc, :], in_=tps,
                                         func=AF.Copy)

            # normalize att for the attention output (off the PE path)
            for ic in range(NC_):
                nc.vector.tensor_scalar(out=att_sb[:, ic, :],
                                        in0=att_sb[:, ic, :],
                                        scalar1=rden[:, ic:ic + 1], scalar2=None,
                                        op0=ALU.mult)
            nc.scalar.dma_start(
                out=att_d[b].rearrange("(k p) j -> p k j", p=P), in_=att_sb)

            # context^T [m%128, m-chunk, i] = v^T E^T (denominator deferred)
            ctxT_sb = work1.tile([P, NM, C], f32r, name="ctxT_sb")
            for mc in range(NM):
                cps = ps2.tile([P, C], f32, tag="ctx", name="cps")
                for jc in range(NC_):
                    nc.tensor.matmul(cps,
                                     lhsT=v_sb[:, jc, mc * P:(mc + 1) * P],
                                     rhs=attT_sb[:, jc, :],
                                     start=(jc == 0), stop=(jc == NC_ - 1))
                nc.scalar.activation(out=ctxT_sb[:, mc, :], in_=cps,
                                     func=AF.Copy)

            # out projection (natural [i%128, s]; one PSUM bank, 4 slices);
            # epilogue applies softmax denominator + bf2 (= bf + Wf@bv)
            ops = ps2.tile([P, NC_ * S], f32, tag="ctx", name="ops")
            for ic in range(NC_):
                for mc in range(NM):
                    nc.tensor.matmul(ops[:, ic * S:(ic + 1) * S],
                                     lhsT=ctxT_sb[:, mc, ic * P:(ic + 1) * P],
                                     rhs=WfT[:, mc, :],
                                     start=(mc == 0), stop=(mc == NM - 1))
            for ic in range(NC_):
                nc.vector.scalar_tensor_tensor(out=hold[:, ic, b, :],
                                               in0=ops[:, ic * S:(ic + 1) * S],
                                               scalar=rden[:, ic:ic + 1],
                                               in1=bf_bc,
                                               op0=ALU.mult, op1=ALU.add)
                nc.vector.bn_stats(out=stats[:, ic, b, :],
                                   in_=hold[:, ic, b, :])

        # vnat load (needed only by the finale; emitted late so it never
        # delays the input-group loads on the sync ring)
        vnat_r = vnat_d.rearrange("b (k p) s -> p k b s", p=P)
        for ic in range(NC_):
            nc.sync.dma_start(out=vnat[:, ic, :, :], in_=vnat_r[:, ic, :, :])

        # ---- sync BN ----
        mv = fin.tile([P, NC_, 2], f32)
        for ic in range(NC_):
            nc.vector.bn_aggr(out=mv[:, ic, :], in_=stats[:, ic, :, :])
        sums = fin.tile([P, 2, NC_], f32)
        tmp = fin.tile([P, NC_], f32)
        # sums[0] = mean * n_local ; sums[1] = (var + mean^2) * n_local
        n_local = float(bpc * S)
        nc.vector.tensor_scalar(out=sums[:, 0, :], in0=mv[:, :, 0],
                                scalar1=n_local, scalar2=None, op0=ALU.mult)
        nc.vector.tensor_mul(tmp, mv[:, :, 0], mv[:, :, 0])
        nc.vector.tensor_add(tmp, tmp, mv[:, :, 1])
        nc.vector.tensor_scalar(out=sums[:, 1, :], in0=tmp,
                                scalar1=n_local, scalar2=None, op0=ALU.mult)

        gsums = fin.tile([P, 2, NC_], f32)
        if use_cc:
            cc_in = dram.tile([P, 2 * NC_], f32)
            cc_out = dram.tile([P, 2 * NC_], f32)
            nc.gpsimd.dma_start(out=cc_in, in_=sums.rearrange("p a k -> p (a k)"))
            nc.gpsimd.collective_compute(
                "AllReduce", ALU.add,
                replica_groups=[list(range(n_cores))],
                ins=[cc_in.opt()], outs=[cc_out.opt()])
            nc.gpsimd.dma_start(out=gsums.rearrange("p a k -> p (a k)"),
                                in_=cc_out)
        else:
            nc.vector.tensor_copy(out=gsums, in_=sums)

        n_glob = float(BN_N) if use_cc else n_local
        mean_g = fin.tile([P, NC_], f32)
        ex2 = fin.tile([P, NC_], f32)
        var_g = fin.tile([P, NC_], f32)
        rstd = fin.tile([P, NC_], f32)
        a_sc = fin.tile([P, NC_], f32)
        b_sc = fin.tile([P, NC_], f32)
        nc.vector.tensor_scalar(out=mean_g, in0=gsums[:, 0, :],
                                scalar1=1.0 / n_glob, scalar2=None, op0=ALU.mult)
        nc.vector.tensor_scalar(out=ex2, in0=gsums[:, 1, :],
                                scalar1=1.0 / n_glob, scalar2=None, op0=ALU.mult)
        nc.vector.tensor_mul(var_g, mean_g, mean_g)
        nc.vector.tensor_sub(var_g, ex2, var_g)
        eps_t = fin.tile([P, 1], f32)
        nc.vector.memset(eps_t, BN_EPS)
        nc.scalar.activation(out=rstd, in_=var_g, func=AF.Sqrt, bias=eps_t)
        nc.vector.reciprocal(out=rstd, in_=rstd)
        nc.vector.tensor_mul(a_sc, gamma_r, rstd)
        nc.vector.tensor_mul(b_sc, mean_g, a_sc)
        nc.vector.tensor_sub(b_sc, beta_r, b_sc)

        # ---- finalize: relu(x*A + B) + value, then store ----
        for ic in range(NC_):
            nc.scalar.activation(out=hold[:, ic, :, :], in_=hold[:, ic, :, :],
                                 func=AF.Relu, scale=a_sc[:, ic:ic + 1],
                                 bias=b_sc[:, ic:ic + 1])
            nc.vector.tensor_add(hold[:, ic, :, :], hold[:, ic, :, :],
                                 vnat[:, ic, :, :])
        out_r = out_d.rearrange("b (k p) s -> p k b s", p=P)
        for ic in range(NC_):
            nc.scalar.dma_start(out=out_r[:, ic, :, :], in_=hold[:, ic, :, :])
        if internal_io:
            nc.scalar.dma_start(out=dummy_d[:, :], in_=hold[:, 0, 0, :])

    nc.compile()
    return nc


def _prep_host(inputs):
    """Host-side sharding prep: transposes + small-param relayouts."""
    q = np.ascontiguousarray(inputs["query"], dtype=np.float32)
    k = np.ascontiguousarray(inputs["key"], dtype=np.float32)
    v = np.ascontiguousarray(inputs["value"], dtype=np.float32)
    qT = np.ascontiguousarray(q.transpose(0, 2, 1))
    kT = np.ascontiguousarray(k.transpose(0, 2, 1))
    vT = np.ascontiguousarray(v.transpose(0, 2, 1))
    WqT = np.ascontiguousarray(np.asarray(inputs["Wq"], np.float32).T)
    WkT = np.ascontiguousarray(np.asarray(inputs["Wk"], np.float32).T)
    WvT = np.ascontiguousarray(np.asarray(inputs["Wv"], np.float32).T)
    WfT = np.ascontiguousarray(np.asarray(inputs["Wf"], np.float32).T)
    bq_r = np.ascontiguousarray(
        np.asarray(inputs["bq"], np.float32).reshape(NM, P).T)
    gamma_r = np.ascontiguousarray(
        np.asarray(inputs["gamma"], np.float32).reshape(NC_, P).T)
    beta_r = np.ascontiguousarray(
        np.asarray(inputs["beta"], np.float32).reshape(NC_, P).T)
    # bv folded through attention (rows sum to 1): bf2 = bf + Wf @ bv
    Wf = np.asarray(inputs["Wf"], np.float32)
    bv = np.asarray(inputs["bv"], np.float32)
    bf2 = np.ascontiguousarray(
        np.asarray(inputs["bf"], np.float32) + Wf @ bv)
    shared = dict(WqT=WqT, WkT=WkT, WvT=WvT, WfT=WfT, bq_r=bq_r,
                  bf2=bf2, gamma_r=gamma_r, beta_r=beta_r)
    in_maps = []
    for c in range(N_CORES):
        sl = slice(c * BPC, (c + 1) * BPC)
        m = dict(shared)
        m["qT"] = qT[sl]
        m["kT"] = kT[sl]
        m["vT"] = vT[sl]
        m["vnat"] = v[sl]
        in_maps.append(m)
    return in_maps


def kernel(**inputs):
    if "nc" not in _CACHE:
        _CACHE["nc"] = build_program()
    nc = _CACHE["nc"]
    in_maps = _prep_host(inputs)
    trace = bool(int(os.environ.get("KERNEL_TRACE", "0")))
    res = run_bass_kernel_spmd(nc, in_maps, list(range(N_CORES)), trace=trace)
    _CACHE["last_result"] = res
    out_full = np.concatenate([r["out_o"] for r in res.results], axis=0)
    att_full = np.concatenate([r["att_o"] for r in res.results], axis=0)
    return out_full, att_full
